# revision 10
# baseline (speedup 1.0000x reference)
"""Trainium2 Bass kernel for nn_MetaBaseline (global-cosine + DN4 few-shot scoring).

Math (per episode b):
  global: logits[q,k] = <qmean_hat, bmean_hat>          (means over the 5x5 spatial grid)
  DN4:    sim[q,p,k,l] = <q_patch[q,p], s_col_hat[k,l]>  -> sum of top-neighbor_k over l,
          summed over p, / neighbor_k
  out = r0 * logits + r1 * dn4

Device strategy (data-parallel, 8 episodes per NeuronCore):
  - host pre-normalizes everything and folds the scalar weights in:
    support columns s_hat (125 per episode/d-tile), class means bm_hat*r0
    (5 extra cols), query patches q_hat, query means qm_hat; the DN4
    patch->query aggregation matrix amat carries r1/neighbor_k.
  - all device tensors are laid out partition-major per episode so each
    input is ONE contiguous dma_start per episode.
  - PE: per episode, 15 qp-tiles x 5 d-tiles of [128,128]x[128,125] bf16
    matmuls -> sim in PSUM (2 qp-tiles share a PSUM bank tile).
  - Scalar/GpSimd alternate on the paired PSUM->SBUF bf16 copies.
  - DVE Max8 per (qp-tile, way) gives top-8 of each 25-value group;
    GpSimd reduce_sum of the first neighbor_k -> draw [128qp, 75].
  - PE aggregation: one PSUM [way, nq] accumulates 15 DN4 matmuls
    (draw^T contracted against amat) plus 5 global matmuls
    (bm_hat*r0 contracted against qm_hat over d) -> final episode scores.
  - host just reshapes/transposes the f32 result.
"""
import numpy as np
import ml_dtypes

N_CORES = 8
B, WAY, SHOT, D, H, W = 64, 5, 1, 640, 5, 5
NQ = 75
HW = H * W                 # 25
QP = NQ * HW               # 1875 query patches per episode
NT = 15                    # qp tiles of 128
QP_PAD = NT * 128          # 1920
ND = D // 128              # 5 contraction tiles
EPC = B // N_CORES         # 8 episodes per core
SCOLS = WAY * HW + WAY     # 130 (125 support cols + 5 class means)
GEPS = 1e-12               # eps of the global-cosine branch (torch F.normalize)

_CACHE = {}
_LAST_IN_MAPS = None


def _build(k: int):
    """Build + compile the SPMD NEFF for top-k = k (k <= 8)."""
    import concourse.bacc as bacc
    import concourse.mybir as mybir
    import concourse.tile as tile

    bf16 = mybir.dt.bfloat16
    f32 = mybir.dt.float32
    COPY = mybir.ActivationFunctionType.Copy

    nc = bacc.Bacc("TRN2", target_bir_lowering=False, debug=False)
    qm = nc.dram_tensor("qm", [EPC, 128, NT * ND * 128], bf16, kind="ExternalInput")
    se = nc.dram_tensor("se", [EPC, 128, ND * SCOLS], bf16, kind="ExternalInput")
    qmh = nc.dram_tensor("qmh", [EPC, 128, ND * NQ], bf16, kind="ExternalInput")
    amat = nc.dram_tensor("amat", [128, NT * NQ], bf16, kind="ExternalInput")
    out = nc.dram_tensor("out", [EPC, WAY, NQ], f32, kind="ExternalOutput")

    with tile.TileContext(nc) as tc:
        with (
            tc.tile_pool(name="const", bufs=1) as cpool,
            tc.tile_pool(name="q", bufs=3) as qpool,
            tc.tile_pool(name="qmh", bufs=3) as qmhpool,
            tc.tile_pool(name="simps", bufs=4, space="PSUM") as simpool,
            tc.tile_pool(name="acc", bufs=2, space="PSUM") as accpool,
            tc.tile_pool(name="simsb", bufs=6) as sbpool,
            tc.tile_pool(name="out8", bufs=2) as o8pool,
            tc.tile_pool(name="draw", bufs=2) as drpool,
            tc.tile_pool(name="osb", bufs=2) as opool,
        ):
            se_t = cpool.tile([128, EPC * ND * SCOLS], bf16)
            amat_t = cpool.tile([128, NT * NQ], bf16)
            qts = [qpool.tile([128, NT * ND * 128], bf16, tag=f"qt{i}",
                              name=f"qt{i}") for i in range(3)]
            qmhs = [qmhpool.tile([128, ND * NQ], bf16, tag=f"qmh{i}",
                                 name=f"qmh{i}") for i in range(3)]

            W_EP = ND * SCOLS           # se cols per episode
            C_EP = NT * ND * 128        # qm cols per episode

            # ---- prologue DMAs: tiny first chunks lead each HWDGE ring so
            # the first matmul chain starts ~4-5us in; episode 1 prefetches
            # right behind them.
            CT = ND * 128  # qm cols per qp-tile
            nc.sync.dma_start(qts[0][:, 0:CT], qm[0, :, 0:CT])
            nc.scalar.dma_start(se_t[:, 0:W_EP], se[0])
            nc.sync.dma_start(qts[0][:, CT:4 * CT], qm[0, :, CT:4 * CT])
            nc.scalar.dma_start(qts[0][:, 4 * CT:9 * CT], qm[0, :, 4 * CT:9 * CT])
            nc.sync.dma_start(qts[0][:, 9 * CT:C_EP], qm[0, :, 9 * CT:C_EP])
            nc.scalar.dma_start(qts[1][:], qm[1])
            nc.gpsimd.dma_start(qmhs[0][:], qmh[0])
            nc.gpsimd.dma_start(amat_t[:], amat[:])

            pending = []  # deferred tail: (e, draw_t, acc_ps)

            def emit_agg(e, draw_t):
                """DN4 + global aggregation matmuls for episode e -> one PSUM."""
                acc = accpool.tile([WAY, NQ], f32, tag="acc")
                for t in range(NT):
                    nc.tensor.matmul(
                        acc[:], draw_t[:, t * WAY:(t + 1) * WAY],
                        amat_t[:, t * NQ:(t + 1) * NQ],
                        start=(t == 0), stop=False,
                    )
                for d in range(ND):
                    off = (e * ND + d) * SCOLS
                    nc.tensor.matmul(
                        acc[:], se_t[:, off + WAY * HW:off + SCOLS],
                        qmhs[e % 3][:, d * NQ:(d + 1) * NQ],
                        start=False, stop=(d == ND - 1),
                    )
                osb = opool.tile([WAY, NQ], f32)
                nc.scalar.activation(osb[:], acc[:], COPY)
                nc.sync.dma_start(out[e], osb[:])

            groups = [(2 * i, min(2 * i + 2, NT)) for i in range((NT + 1) // 2)]
            for e in range(EPC):
                qt = qts[e % 3]
                out8 = o8pool.tile([128, NT * WAY * 8], bf16)
                for gi, (t0, t1) in enumerate(groups):
                    simps = simpool.tile([128, 250], f32, tag="simps")
                    for t in range(t0, t1):
                        off = (t - t0) * WAY * HW
                        for d in range(ND):
                            nc.tensor.matmul(
                                simps[:, off:off + WAY * HW],
                                qt[:, (t * ND + d) * 128:(t * ND + d + 1) * 128],
                                se_t[:, (e * ND + d) * SCOLS:(e * ND + d) * SCOLS + WAY * HW],
                                start=(d == 0), stop=(d == ND - 1),
                            )
                    w = (t1 - t0) * WAY * HW
                    simsb = sbpool.tile([128, 250], bf16)
                    nc.scalar.activation(simsb[:, 0:w], simps[:, 0:w], COPY)
                    for t in range(t0, t1):
                        off = (t - t0) * WAY * HW
                        for kk in range(WAY):
                            g = t * WAY + kk
                            nc.vector.max(
                                out8[:, g * 8:(g + 1) * 8],
                                simsb[:, off + kk * HW:off + (kk + 1) * HW],
                            )
                    # prefetch + deferred aggregation, spread across the episode
                    if gi == 0:
                        if e + 2 < EPC:  # 2-episode DMA lead, alternating rings
                            eng2 = nc.sync if e % 2 == 0 else nc.scalar
                            eng2.dma_start(qts[(e + 2) % 3][:], qm[e + 2])
                    elif gi == 1:
                        if pending:
                            emit_agg(*pending.pop())
                    elif gi == 2:
                        if e + 1 < EPC:
                            nc.scalar.dma_start(
                                se_t[:, (e + 1) * W_EP:(e + 2) * W_EP], se[e + 1])
                            nc.gpsimd.dma_start(qmhs[(e + 1) % 3][:], qmh[e + 1])
                draw_t = drpool.tile([128, NT * WAY], bf16)
                o8v = out8[:].rearrange("p (g e) -> p g e", e=8)
                with nc.allow_low_precision("bf16 top-k sums feed a bf16 matmul"):
                    if e == EPC - 1:
                        # tail: DVE is idle here and much faster than the
                        # Pool add-chain; shortens the final agg's wait
                        nc.vector.reduce_sum(
                            draw_t[:], o8v[:, :, 0:k], axis=mybir.AxisListType.X)
                    elif k == 1:
                        nc.gpsimd.tensor_copy(draw_t[:], o8v[:, :, 0])
                    else:
                        nc.gpsimd.tensor_add(draw_t[:], o8v[:, :, 0], o8v[:, :, 1])
                        for j in range(2, k):
                            nc.gpsimd.tensor_add(draw_t[:], draw_t[:], o8v[:, :, j])
                pending.append((e, draw_t))
            emit_agg(*pending.pop())
    nc.compile()
    return nc


def kernel(base, query, r, neighbor_k):
    from concourse.bass_utils import run_bass_kernel_spmd

    k = int(neighbor_k)
    assert 1 <= k <= 8, f"top-k must fit the Max8 output, got {k}"
    base = np.asarray(base, dtype=np.float32).reshape(B, WAY, D, HW)
    query = np.asarray(query, dtype=np.float32).reshape(B, NQ, D, HW)
    r = np.asarray(r, dtype=np.float32)

    # ---- host prep (layout + normalization, scalar weights folded in) ----
    # support: normalized columns + r0-scaled normalized class means
    s_norm = base / np.linalg.norm(base, axis=2, keepdims=True)
    bmean = base.mean(axis=3)                                     # [B, way, D]
    bm = bmean / np.maximum(np.linalg.norm(bmean, axis=2, keepdims=True), GEPS)
    s_ext = np.empty((B, D, SCOLS), dtype=np.float32)
    s_ext[:, :, :WAY * HW] = s_norm.transpose(0, 2, 1, 3).reshape(B, D, WAY * HW)
    s_ext[:, :, WAY * HW:] = (r[0] * bm).transpose(0, 2, 1)
    # -> [B, 128, ND*SCOLS] partition-major (one dma per episode)
    s_ext = s_ext.reshape(B, ND, 128, SCOLS).transpose(0, 2, 1, 3)
    s_ext = np.ascontiguousarray(s_ext.reshape(B, 128, ND * SCOLS)).astype(ml_dtypes.bfloat16)

    # query patches: normalized, [B, 128, NT*ND*128] (tile-major free dim)
    qn = np.sqrt(np.einsum("bqdp,bqdp->bqp", query, query))      # [B, nq, hw]
    q_hat = query / qn[:, :, None, :]
    q_mat = np.zeros((B, D, QP_PAD), dtype=np.float32)
    q_mat[:, :, :QP] = q_hat.transpose(0, 2, 1, 3).reshape(B, D, QP)
    q_mat = q_mat.reshape(B, ND, 128, NT, 128).transpose(0, 2, 3, 1, 4)
    q_mat = np.ascontiguousarray(
        q_mat.reshape(B, 128, NT * ND * 128)).astype(ml_dtypes.bfloat16)

    # query means: normalized, [B, 128, ND*NQ]
    qmean = query.mean(axis=3)                                    # [B, nq, D]
    qmh = qmean / np.maximum(np.linalg.norm(qmean, axis=2, keepdims=True), GEPS)
    qmh = qmh.transpose(0, 2, 1).reshape(B, ND, 128, NQ).transpose(0, 2, 1, 3)
    qmh = np.ascontiguousarray(qmh.reshape(B, 128, ND * NQ)).astype(ml_dtypes.bfloat16)

    # patch->query aggregation matrix (r1/k folded), [128, NT*NQ]
    am = np.zeros((128, NT, NQ), dtype=np.float32)
    for t in range(NT):
        qp_idx = t * 128 + np.arange(128)
        valid = qp_idx < QP
        am[valid, t, qp_idx[valid] // HW] = r[1] / k
    am = am.reshape(128, NT * NQ).astype(ml_dtypes.bfloat16)

    if k not in _CACHE:
        _CACHE[k] = _build(k)
    nc = _CACHE[k]

    in_maps = []
    for c in range(N_CORES):
        sl = slice(c * EPC, (c + 1) * EPC)
        in_maps.append({
            "qm": q_mat[sl],
            "se": s_ext[sl],
            "qmh": qmh[sl],
            "amat": am,
        })
    global _LAST_IN_MAPS
    _LAST_IN_MAPS = in_maps
    res = run_bass_kernel_spmd(nc, in_maps, list(range(N_CORES)))
    dev = np.stack([res.results[c]["out"] for c in range(N_CORES)])  # [C, EPC, WAY, NQ]
    return np.ascontiguousarray(
        dev.reshape(B, WAY, NQ).transpose(0, 2, 1)).astype(np.float32)


# revision 11
# speedup vs baseline: 1.0604x; 1.0604x over previous
"""Trainium2 Bass kernel for nn_MetaBaseline (global-cosine + DN4 few-shot scoring).

Math (per episode b):
  global: logits[q,k] = <qmean_hat, bmean_hat>          (means over the 5x5 spatial grid)
  DN4:    sim[q,p,k,l] = <q_patch[q,p], s_col_hat[k,l]>  -> sum of top-neighbor_k over l,
          summed over p, / neighbor_k
  out = r0 * logits + r1 * dn4

Device strategy (data-parallel, 8 episodes per NeuronCore):
  - host pre-normalizes everything and folds the scalar weights in:
    support columns s_hat (125 per episode/d-tile), class means bm_hat*r0
    (5 extra cols), query patches q_hat, query means qm_hat; the DN4
    patch->query aggregation matrix amat carries r1/neighbor_k.
  - all device tensors are laid out partition-major per episode so each
    input is ONE contiguous dma_start per episode.
  - PE: per episode, 15 qp-tiles x 5 d-tiles of [128,128]x[128,125] bf16
    matmuls -> sim in PSUM (2 qp-tiles share a PSUM bank tile).
  - Scalar/GpSimd alternate on the paired PSUM->SBUF bf16 copies.
  - DVE Max8 per (qp-tile, way) gives top-8 of each 25-value group;
    GpSimd reduce_sum of the first neighbor_k -> draw [128qp, 75].
  - PE aggregation: one PSUM [way, nq] accumulates 15 DN4 matmuls
    (draw^T contracted against amat) plus 5 global matmuls
    (bm_hat*r0 contracted against qm_hat over d) -> final episode scores.
  - host just reshapes/transposes the f32 result.
"""
import numpy as np
import ml_dtypes

N_CORES = 8
B, WAY, SHOT, D, H, W = 64, 5, 1, 640, 5, 5
NQ = 75
HW = H * W                 # 25
QP = NQ * HW               # 1875 query patches per episode
NT = 15                    # qp tiles of 128
QP_PAD = NT * 128          # 1920
ND = D // 128              # 5 contraction tiles
EPC = B // N_CORES         # 8 episodes per core
SCOLS = WAY * HW + WAY     # 130 (125 support cols + 5 class means)
GEPS = 1e-12               # eps of the global-cosine branch (torch F.normalize)

_CACHE = {}
_LAST_IN_MAPS = None


def _build(k: int):
    """Build + compile the SPMD NEFF for top-k = k (k <= 8)."""
    import concourse.bacc as bacc
    import concourse.mybir as mybir
    import concourse.tile as tile

    bf16 = mybir.dt.bfloat16
    f32 = mybir.dt.float32
    COPY = mybir.ActivationFunctionType.Copy

    nc = bacc.Bacc("TRN2", target_bir_lowering=False, debug=False)
    qm = nc.dram_tensor("qm", [EPC, 128, NT * ND * 128], bf16, kind="ExternalInput")
    se = nc.dram_tensor("se", [EPC, 128, ND * SCOLS], bf16, kind="ExternalInput")
    qmh = nc.dram_tensor("qmh", [EPC, 128, ND * NQ], bf16, kind="ExternalInput")
    amat = nc.dram_tensor("amat", [128, NT * NQ], bf16, kind="ExternalInput")
    out = nc.dram_tensor("out", [EPC, WAY, NQ], f32, kind="ExternalOutput")

    with tile.TileContext(nc) as tc:
        with (
            tc.tile_pool(name="const", bufs=1) as cpool,
            tc.tile_pool(name="q", bufs=3) as qpool,
            tc.tile_pool(name="qmh", bufs=3) as qmhpool,
            tc.tile_pool(name="simps", bufs=4, space="PSUM") as simpool,
            tc.tile_pool(name="acc", bufs=2, space="PSUM") as accpool,
            tc.tile_pool(name="simsb", bufs=6) as sbpool,
            tc.tile_pool(name="out8", bufs=2) as o8pool,
            tc.tile_pool(name="draw", bufs=2) as drpool,
            tc.tile_pool(name="osb", bufs=2) as opool,
        ):
            se_t = cpool.tile([128, EPC * ND * SCOLS], bf16)
            amat_t = cpool.tile([128, NT * NQ], bf16)
            qts = [qpool.tile([128, NT * ND * 128], bf16, tag=f"qt{i}",
                              name=f"qt{i}") for i in range(3)]
            qmhs = [qmhpool.tile([128, ND * NQ], bf16, tag=f"qmh{i}",
                                 name=f"qmh{i}") for i in range(3)]

            W_EP = ND * SCOLS           # se cols per episode
            C_EP = NT * ND * 128        # qm cols per episode

            # ---- prologue DMAs: tiny first chunks lead each HWDGE ring so
            # the first matmul chain starts ~4-5us in; episode 1 prefetches
            # right behind them.
            CT = ND * 128  # qm cols per qp-tile
            nc.sync.dma_start(qts[0][:, 0:CT], qm[0, :, 0:CT])
            nc.scalar.dma_start(se_t[:, 0:W_EP], se[0])
            nc.sync.dma_start(qts[0][:, CT:4 * CT], qm[0, :, CT:4 * CT])
            nc.scalar.dma_start(qts[0][:, 4 * CT:9 * CT], qm[0, :, 4 * CT:9 * CT])
            nc.sync.dma_start(qts[0][:, 9 * CT:C_EP], qm[0, :, 9 * CT:C_EP])
            nc.scalar.dma_start(qts[1][:], qm[1])
            nc.gpsimd.dma_start(qmhs[0][:], qmh[0])
            nc.gpsimd.dma_start(amat_t[:], amat[:])

            pending = []  # deferred tail: (e, draw_t, acc_ps)

            def emit_agg(e, draw_t):
                """DN4 + global aggregation matmuls for episode e -> one PSUM."""
                acc = accpool.tile([WAY, NQ], f32, tag="acc")
                for t in range(NT):
                    nc.tensor.matmul(
                        acc[:], draw_t[:, t * WAY:(t + 1) * WAY],
                        amat_t[:, t * NQ:(t + 1) * NQ],
                        start=(t == 0), stop=False,
                    )
                for d in range(ND):
                    off = (e * ND + d) * SCOLS
                    nc.tensor.matmul(
                        acc[:], se_t[:, off + WAY * HW:off + SCOLS],
                        qmhs[e % 3][:, d * NQ:(d + 1) * NQ],
                        start=False, stop=(d == ND - 1),
                    )
                osb = opool.tile([WAY, NQ], f32)
                nc.scalar.activation(osb[:], acc[:], COPY)
                nc.gpsimd.dma_start(out[e], osb[:])

            groups = [(2 * i, min(2 * i + 2, NT)) for i in range((NT + 1) // 2)]
            for e in range(EPC):
                qt = qts[e % 3]
                out8 = o8pool.tile([128, NT * WAY * 8], bf16)
                for gi, (t0, t1) in enumerate(groups):
                    simps = simpool.tile([128, 250], f32, tag="simps")
                    for t in range(t0, t1):
                        off = (t - t0) * WAY * HW
                        for d in range(ND):
                            nc.tensor.matmul(
                                simps[:, off:off + WAY * HW],
                                qt[:, (t * ND + d) * 128:(t * ND + d + 1) * 128],
                                se_t[:, (e * ND + d) * SCOLS:(e * ND + d) * SCOLS + WAY * HW],
                                start=(d == 0), stop=(d == ND - 1),
                            )
                    w = (t1 - t0) * WAY * HW
                    simsb = sbpool.tile([128, 250], bf16)
                    nc.scalar.activation(simsb[:, 0:w], simps[:, 0:w], COPY)
                    for t in range(t0, t1):
                        off = (t - t0) * WAY * HW
                        for kk in range(WAY):
                            g = t * WAY + kk
                            nc.vector.max(
                                out8[:, g * 8:(g + 1) * 8],
                                simsb[:, off + kk * HW:off + (kk + 1) * HW],
                            )
                    # prefetch + deferred aggregation, spread across the episode
                    if gi == 0:
                        if e + 2 < EPC:  # 2-episode DMA lead, alternating rings
                            eng2 = nc.sync if e % 2 == 0 else nc.scalar
                            eng2.dma_start(qts[(e + 2) % 3][:], qm[e + 2])
                    elif gi == 1:
                        if pending:
                            emit_agg(*pending.pop())
                    elif gi == 2:
                        if e + 1 < EPC:
                            nc.scalar.dma_start(
                                se_t[:, (e + 1) * W_EP:(e + 2) * W_EP], se[e + 1])
                            nc.gpsimd.dma_start(qmhs[(e + 1) % 3][:], qmh[e + 1])
                draw_t = drpool.tile([128, NT * WAY], bf16)
                o8v = out8[:].rearrange("p (g e) -> p g e", e=8)
                with nc.allow_low_precision("bf16 top-k sums feed a bf16 matmul"):
                    if e == EPC - 1:
                        # tail: DVE is idle here and much faster than the
                        # Pool add-chain; shortens the final agg's wait
                        nc.vector.reduce_sum(
                            draw_t[:], o8v[:, :, 0:k], axis=mybir.AxisListType.X)
                    elif k == 1:
                        nc.gpsimd.tensor_copy(draw_t[:], o8v[:, :, 0])
                    else:
                        nc.gpsimd.tensor_add(draw_t[:], o8v[:, :, 0], o8v[:, :, 1])
                        for j in range(2, k):
                            nc.gpsimd.tensor_add(draw_t[:], draw_t[:], o8v[:, :, j])
                pending.append((e, draw_t))
            emit_agg(*pending.pop())
    nc.compile()
    return nc


def kernel(base, query, r, neighbor_k):
    from concourse.bass_utils import run_bass_kernel_spmd

    k = int(neighbor_k)
    assert 1 <= k <= 8, f"top-k must fit the Max8 output, got {k}"
    base = np.asarray(base, dtype=np.float32).reshape(B, WAY, D, HW)
    query = np.asarray(query, dtype=np.float32).reshape(B, NQ, D, HW)
    r = np.asarray(r, dtype=np.float32)

    # ---- host prep (layout + normalization, scalar weights folded in) ----
    # support: normalized columns + r0-scaled normalized class means
    s_norm = base / np.linalg.norm(base, axis=2, keepdims=True)
    bmean = base.mean(axis=3)                                     # [B, way, D]
    bm = bmean / np.maximum(np.linalg.norm(bmean, axis=2, keepdims=True), GEPS)
    s_ext = np.empty((B, D, SCOLS), dtype=np.float32)
    s_ext[:, :, :WAY * HW] = s_norm.transpose(0, 2, 1, 3).reshape(B, D, WAY * HW)
    s_ext[:, :, WAY * HW:] = (r[0] * bm).transpose(0, 2, 1)
    # -> [B, 128, ND*SCOLS] partition-major (one dma per episode)
    s_ext = s_ext.reshape(B, ND, 128, SCOLS).transpose(0, 2, 1, 3)
    s_ext = np.ascontiguousarray(s_ext.reshape(B, 128, ND * SCOLS)).astype(ml_dtypes.bfloat16)

    # query patches: normalized, [B, 128, NT*ND*128] (tile-major free dim)
    qn = np.sqrt(np.einsum("bqdp,bqdp->bqp", query, query))      # [B, nq, hw]
    q_hat = query / qn[:, :, None, :]
    q_mat = np.zeros((B, D, QP_PAD), dtype=np.float32)
    q_mat[:, :, :QP] = q_hat.transpose(0, 2, 1, 3).reshape(B, D, QP)
    q_mat = q_mat.reshape(B, ND, 128, NT, 128).transpose(0, 2, 3, 1, 4)
    q_mat = np.ascontiguousarray(
        q_mat.reshape(B, 128, NT * ND * 128)).astype(ml_dtypes.bfloat16)

    # query means: normalized, [B, 128, ND*NQ]
    qmean = query.mean(axis=3)                                    # [B, nq, D]
    qmh = qmean / np.maximum(np.linalg.norm(qmean, axis=2, keepdims=True), GEPS)
    qmh = qmh.transpose(0, 2, 1).reshape(B, ND, 128, NQ).transpose(0, 2, 1, 3)
    qmh = np.ascontiguousarray(qmh.reshape(B, 128, ND * NQ)).astype(ml_dtypes.bfloat16)

    # patch->query aggregation matrix (r1/k folded), [128, NT*NQ]
    am = np.zeros((128, NT, NQ), dtype=np.float32)
    for t in range(NT):
        qp_idx = t * 128 + np.arange(128)
        valid = qp_idx < QP
        am[valid, t, qp_idx[valid] // HW] = r[1] / k
    am = am.reshape(128, NT * NQ).astype(ml_dtypes.bfloat16)

    if k not in _CACHE:
        _CACHE[k] = _build(k)
    nc = _CACHE[k]

    in_maps = []
    for c in range(N_CORES):
        sl = slice(c * EPC, (c + 1) * EPC)
        in_maps.append({
            "qm": q_mat[sl],
            "se": s_ext[sl],
            "qmh": qmh[sl],
            "amat": am,
        })
    global _LAST_IN_MAPS
    _LAST_IN_MAPS = in_maps
    res = run_bass_kernel_spmd(nc, in_maps, list(range(N_CORES)))
    dev = np.stack([res.results[c]["out"] for c in range(N_CORES)])  # [C, EPC, WAY, NQ]
    return np.ascontiguousarray(
        dev.reshape(B, WAY, NQ).transpose(0, 2, 1)).astype(np.float32)
